# revision 6
# baseline (speedup 1.0000x reference)
import sys

sys.path.insert(0, "/opt/trn_rl_repo")
import numpy as np

# --- Problem geometry (hardcoded from the nn_DifferentiableBackprojection spec) ---
B, C, A, V, U = 1, 8, 120, 128, 128
NZ, NY, NX = 96, 96, 96
DSO = 1000.0
DSD = 1500.0
DU = DV = 1.0
DVOX = 0.8
NYX = NY * NX  # 9216
KB = 8  # v-band taps per z (covers max band width)
N_CORES = 8

_prog_cache = {}


def _geom_jax(angles):
    """iu [A, NYX], iv [A, NZ, NYX] in fp32, computed with jax on CPU using the
    exact op sequence of the reference (so floor() knife-edges agree)."""
    import jax
    import jax.numpy as jnp

    cpu = jax.devices("cpu")[0]

    @jax.jit
    def geom(angles):
        z = (jnp.arange(NZ, dtype=jnp.float32) - (NZ - 1) / 2.0) * DVOX
        y = (jnp.arange(NY, dtype=jnp.float32) - (NY - 1) / 2.0) * DVOX
        x = (jnp.arange(NX, dtype=jnp.float32) - (NX - 1) / 2.0) * DVOX
        zg, yg, xg = z[:, None, None], y[None, :, None], x[None, None, :]

        def one(ang):
            c, s = jnp.cos(ang), jnp.sin(ang)
            xr = xg * c + yg * s
            yr = -xg * s + yg * c
            dist = DSO - xr
            mag = DSD / dist
            iu = jnp.broadcast_to(
                yr * mag / DU + (U - 1) / 2.0, (NZ, NY, NX)
            ).reshape(NZ * NY * NX)[: NY * NX]
            iv = (zg * mag / DV + (V - 1) / 2.0).reshape(NZ, NY * NX)
            w = jnp.broadcast_to(mag * mag, (NZ, NY, NX)).reshape(NZ * NY * NX)[
                : NY * NX
            ]
            return iu, iv, w

        return jax.vmap(one)(angles)

    with jax.default_device(cpu):
        iu, iv, w = geom(jnp.asarray(angles, dtype=jnp.float32))
    return np.asarray(iu), np.asarray(iv), np.asarray(w)


def _host_tables(angles):
    """Per-angle geometry tables, replicating reference.py ops in float32.

    Returns:
      gu:  [A, U, NYX] f16    u-interp hat weights * distance weight * valid
      h:   [A, KB, NZ, NYX] f16   v-interp hat weights
      rows:[A, KB, NZ] int64  sinogram v-row index per tap (clipped)
    """
    f32 = np.float32
    iu_all, iv_all, w_all = _geom_jax(angles)

    gu = np.zeros((A, U, NYX), np.float16)
    h = np.zeros((A, KB, NZ, NYX), np.float16)
    rows = np.zeros((A, KB, NZ), np.int64)

    uu = np.arange(U, dtype=f32)[:, None]  # [U, 1]

    for a in range(A):
        iu = iu_all[a]
        iv = iv_all[a]
        assert iv.min() >= 0.0 and iv.max() <= V - 1, "iv out of range"
        valid = (iu >= 0) & (iu <= U - 1)
        w = w_all[a] * valid.astype(f32)

        # u hats: relu(1 - |u - iu|) * w  == exact bilinear u-weights (valid voxels)
        gu[a] = (
            np.maximum(f32(0.0), f32(1.0) - np.abs(uu - iu[None, :])) * w[None, :]
        ).astype(np.float16)

        v0 = np.floor(iv).astype(np.int64)
        b = v0.min(axis=1)  # [NZ]
        assert int((v0.max(axis=1) - b).max()) <= KB - 2, "band too wide"
        for k in range(KB):
            j = b + k  # [NZ]
            h[a, k] = np.maximum(
                f32(0.0), f32(1.0) - np.abs(iv - j[:, None].astype(f32))
            ).astype(np.float16)
            rows[a, k] = np.clip(j, 0, V - 1)
    return gu, h, rows


def _build_program():
    if "nc" in _prog_cache:
        return _prog_cache["nc"]
    import concourse.bass as bass
    import concourse.tile as tile
    from concourse import mybir, bacc

    FREE = KB * NZ + NYX  # 768 + 9216
    CH = 2048  # mult/add chunk (4 PSUM banks)

    nc = bacc.Bacc("TRN2", target_bir_lowering=False, debug=False)
    segu_d = nc.dram_tensor(
        "segu", (A, U, FREE), mybir.dt.float16, kind="ExternalInput"
    )
    h_d = nc.dram_tensor("h", (A, KB, NZ, NYX), mybir.dt.float16, kind="ExternalInput")
    out_d = nc.dram_tensor("out", (NZ, NYX), mybir.dt.float32, kind="ExternalOutput")

    with tile.TileContext(nc) as tc:
        with (
            tc.tile_pool(name="persist", bufs=1) as pp,
            tc.tile_pool(name="io", bufs=2) as io,
            tc.tile_pool(name="work", bufs=2) as wk,
            tc.tile_pool(name="ps", bufs=2, space=bass.MemorySpace.PSUM) as ps,
        ):
            acc = pp.tile([NZ, NYX], mybir.dt.float32)
            nc.vector.memset(acc[:], 0.0)

            for a in range(A):
                segu = io.tile([U, FREE], mybir.dt.float16, tag="segu")
                nc.gpsimd.dma_start(segu[:], segu_d.ap()[a])
                for k in range(KB):
                    hk = io.tile([NZ, NYX], mybir.dt.float16, tag="hk")
                    nc.gpsimd.dma_start(hk[:], h_d.ap()[a, k])
                    for n0 in range(0, NYX, CH):
                        n = min(CH, NYX - n0)
                        te = ps.tile([NZ, CH], mybir.dt.float32, tag="te")
                        for j in range(0, n, 512):
                            nc.tensor.matmul(
                                te[:, j : j + 512],
                                segu[:, k * NZ : (k + 1) * NZ],
                                segu[:, KB * NZ + n0 + j : KB * NZ + n0 + j + 512],
                                start=True,
                                stop=True,
                            )
                        m = wk.tile([NZ, CH], mybir.dt.float32, tag="m")
                        nc.vector.tensor_mul(m[:, :n], te[:, :n], hk[:, n0 : n0 + n])
                        nc.vector.tensor_add(
                            acc[:, n0 : n0 + n], acc[:, n0 : n0 + n], m[:, :n]
                        )
            nc.sync.dma_start(out_d.ap(), acc[:])
    nc.compile()
    _prog_cache["nc"] = nc
    return nc


def _install_ntff_shim():
    """Provide antenv.axon_hooks (missing in this image) so trace=True works."""
    import types, importlib

    try:
        from antenv.axon_hooks import get_axon_ntff_profile_hook  # noqa: F401

        return True
    except ImportError:
        pass
    try:
        import antenv

        mod = types.ModuleType("antenv.axon_hooks")
        mod._hook = None

        def set_axon_ntff_profile_hook(h):
            mod._hook = h

        def get_axon_ntff_profile_hook():
            return mod._hook

        mod.set_axon_ntff_profile_hook = set_axon_ntff_profile_hook
        mod.get_axon_ntff_profile_hook = get_axon_ntff_profile_hook
        sys.modules["antenv.axon_hooks"] = mod
        antenv.axon_hooks = mod
        if "/root/.axon_site" not in sys.path:
            sys.path.insert(0, "/root/.axon_site")
        boot = importlib.import_module("trn_agent_boot.trn_boot")
        hook = boot._ntff_profile_via_ctypes("/opt/axon/libaxon_pjrt.so")
        if hook is None:
            return False
        mod._hook = hook
        return True
    except Exception as e:  # pragma: no cover
        print(f"ntff shim failed: {e}")
        return False


def kernel(sinogram, angles):
    import os
    from concourse.bass_utils import run_bass_kernel_spmd

    sinogram = np.asarray(sinogram)
    angles = np.asarray(angles)
    in_dtype = sinogram.dtype
    gu, h, rows = _host_tables(angles)

    sino = sinogram.reshape(C, A, V, U).astype(np.float32)
    ai = np.arange(A)[:, None, None]
    in_maps = []
    for c in range(C):
        se = sino[c][ai, rows]  # [A, KB, NZ, U]
        se_t = np.ascontiguousarray(np.transpose(se, (0, 3, 1, 2)))  # [A, U, KB, NZ]
        segu = np.concatenate(
            [se_t.reshape(A, U, KB * NZ).astype(np.float16), gu], axis=2
        )  # [A, U, KB*NZ + NYX]
        in_maps.append({"segu": np.ascontiguousarray(segu), "h": h})

    nc = _build_program()
    trace = bool(os.environ.get("BP_TRACE")) and _install_ntff_shim()
    res = run_bass_kernel_spmd(nc, in_maps, list(range(N_CORES)), trace=trace)
    _prog_cache["last_results"] = res
    vols = np.stack(
        [res.results[i]["out"].reshape(NZ, NY, NX) for i in range(N_CORES)]
    )
    return vols.reshape(B, C, NZ, NY, NX).astype(in_dtype, copy=False)
